# revision 88
# baseline (speedup 1.0000x reference)
"""GQA forward (B=2,T=2048,D=1024,H=16,KV=4,HD=64) on 8 TRN2 NeuronCores.

Sharding: core c -> (batch b=c//4, kv-group g=c%4). Each core computes the
4 query heads of its kv group against its batch, plus the partial output
projection for its 256 columns of the concat-head activation; the host sums
the 4 per-group partials of each batch (row-parallel out_proj unshard).

v4 pipeline (over v3, 241us -> ~190us):
- Host pre-tiles every input to its exact sbuf layout (x as 4 t-slices of
  [128p, 8c, 512t]) so all loads are fully contiguous multi-KB-line DMAs.
- t-sliced head: kv and q(m0) project per 512-col t-slice into one shared
  psum tile (kv cols 0:512, q0 cols 512:1024; the two banks interleaved
  per c-chunk -- consecutive matmuls into the SAME psum bank serialize at
  ~645ns while alternating banks pipeline at ~215ns). head_ts(2)/(3) are
  injected into early attention slices so the PE stream does not
  serialize on the x DMA tail; ~20 dummy warmup matmuls raise the HAM
  clock-gate to 8/8 while x streams in. rope: ACT psum->bf16 stage, DVE
  stream_shuffle + 2x-bf16 mul/mul/add (k's muls on gpsimd); compact
  cos/sin (64,T) loaded on the scalar HWDGE queue, row-duplicated by
  sbuf->sbuf DMA. Attention starts ~25us in instead of ~40us.
- slices processed in PAIRS (exp x2 -> QK-prefetch x2 -> PV x2): halves
  the QK<->PV array-configuration transitions (each costs a ~170ns drain).
  Score ring 3x[128,1024] (6 psum banks) + one [65,1024] PV accumulator
  (2 banks). exp alternates ACT (table exp)/DVE (Schraudolph int16
  bit-trick) within each pair, ~9:7 ratio per unit.
- tq-major unit order ((0,0),(1,0),(0,1),...): each 512-col t-block of ot
  completes after 2 units; its out_proj is interleaved into the next unit
  pair as ring pseudo-slices, leaving only tq=3's out_proj as tail.
  q(m1) projection lumps injected at slices 8 and 36.
- ones-column at v col 64 gives the softmax denominator at psum partition
  64. Norm chain: ACT of-copy (psum reads must start at partition 0!) ->
  DVE-copy den hop to partition 0 -> DVE recip -> gpsimd
  partition_broadcast -> 2 DVE muls (f32xf32->bf16) into ot_sbc.
"""

import os
import sys

for _p in ("/opt/trn_rl_repo",):
    if _p not in sys.path:
        sys.path.insert(0, _p)

import numpy as np

B, T, D = 2, 2048, 1024
H, KV, HD = 16, 4, 64
REP = H // KV          # 4 query heads per core
GH = REP * HD          # 256 q columns per core
P = 128
SC = T // P            # s-chunks (contraction tiles over sequence)
KC = D // P            # k-chunks over the model dim
TS = 4                 # t-slices of 512 for the head

SWAP_MASK = [i ^ 1 for i in range(32)]  # pair-swap within each 32-part quadrant

LOG2E = 1.4426950408889634
# Schraudolph constants for bf16-bits-in-int16: round((x/8)*a' + b') == bf16
# bits of exp(x/8) with max rel err ~3%; attention scale 1/8 folded in.
SCH_A = 128.0 * LOG2E / 8.0
SCH_B = 128.0 * (127.0 - 0.04303)

# exp engine pattern per unit: True = ACT (table exp), False = DVE
# (Schraudolph); pairs (2p, 2p+1) alternate engines so both run concurrently
def _exp_pattern(n_act):
    pat = [s % 2 == 0 for s in range(16)]  # 8:8 alternating
    extra = [7, 13, 3, 9]                  # odd slots flipped to ACT
    for j in range(n_act - 8):
        pat[extra[j]] = True
    return pat

EXP_PAT = [_exp_pattern(n) for n in (10, 10, 9, 9, 9, 9, 9, 9)]

_MODULE_CACHE = {}
LAST_RESULT = None  # test.py reads exec_time_ns / trace path from here


def _build():
    import concourse.tile as tile
    from concourse import mybir
    from concourse.bacc import Bacc

    bf16 = mybir.dt.bfloat16
    f32 = mybir.dt.float32
    i16 = mybir.dt.int16
    AF = mybir.ActivationFunctionType
    ALU = mybir.AluOpType

    nc = Bacc(trn_type="TRN2")
    # all inputs host-pre-tiled to the exact sbuf layouts -> fully
    # contiguous DMAs with multi-KB per-partition lines
    xts_h = nc.dram_tensor("xts", (TS * P, KC * 512), bf16, kind="ExternalInput")
    w01_h = nc.dram_tensor("w01T", (P, 2 * KC * P), bf16, kind="ExternalInput")
    qw1_h = nc.dram_tensor("qw1T", (P, KC * P), bf16, kind="ExternalInput")
    owT_h = nc.dram_tensor("owT", (P, 2 * D), bf16, kind="ExternalInput")
    cosC_h = nc.dram_tensor("cosC", (64, T), bf16, kind="ExternalInput")
    sinC_h = nc.dram_tensor("sinC", (64, T), bf16, kind="ExternalInput")
    out_h = nc.dram_tensor("outT", (D, T), bf16, kind="ExternalOutput")
    debug = os.environ.get("KERNEL_DEBUG", "0") == "1"
    if debug:
        dbg_kd_h = nc.dram_tensor("dbg_kd", (P, 4 * 512), bf16, kind="ExternalOutput")
        dbg_v_h = nc.dram_tensor("dbg_v", (P, 4 * 288), bf16, kind="ExternalOutput")
        dbg_qro_h = nc.dram_tensor("dbg_qro", (P, 8 * 512), bf16,
                                   kind="ExternalOutput")
        dbg_ot_h = nc.dram_tensor("dbg_ot", (P, 8 * 512), bf16, kind="ExternalOutput")

    outr = out_h[:, :].rearrange("(c p) t -> p c t", p=P)

    with tile.TileContext(nc) as tc:
        with (
            tc.tile_pool(name="consts", bufs=1) as consts,
            tc.tile_pool(name="stg", bufs=2) as stg,
            tc.tile_pool(name="rope", bufs=2) as rope,
            tc.tile_pool(name="pexp", bufs=3) as pexp,
            tc.tile_pool(name="psch", bufs=3) as psch,
            tc.tile_pool(name="norm", bufs=2) as norm,
            tc.tile_pool(name="outs", bufs=2) as outs,
            tc.tile_pool(name="ps_sc", bufs=3, space="PSUM") as ps_sc,
            tc.tile_pool(name="ps_ot", bufs=1, space="PSUM") as ps_ot,
        ):
            # ---- loads: big tensors on sync, cos/sin + expansion on scalar ----
            # kvw + qw0 in ONE load: halves the sync-issue time ahead of x
            w01_sb = consts.tile([P, 2, KC, P], bf16)
            nc.sync.dma_start(out=w01_sb,
                              in_=w01_h[:, :].rearrange("p (w c m) -> p w c m",
                                                        w=2, c=KC))
            kvw_sb = w01_sb[:, 0]
            qw0_sb = w01_sb[:, 1]
            cos_sb = consts.tile([P, T], bf16)
            nc.scalar.dma_start(out=cos_sb[0:64, :], in_=cosC_h[:, :])
            sin_sb = consts.tile([P, T], bf16)
            nc.scalar.dma_start(out=sin_sb[0:64, :], in_=sinC_h[:, :])
            x_sb = [consts.tile([P, KC, 512], bf16, tag=f"x{ts}", name=f"x{ts}")
                    for ts in range(TS)]
            for ts in range(TS):
                nc.sync.dma_start(
                    out=x_sb[ts],
                    in_=xts_h[ts * P : (ts + 1) * P, :].rearrange(
                        "p (c t) -> p c t", c=KC),
                )
            qw1_sb = consts.tile([P, KC, P], bf16)
            nc.sync.dma_start(out=qw1_sb,
                              in_=qw1_h[:, :].rearrange("p (c m) -> p c m", c=KC))
            owT_sb = consts.tile([P, 2, D], bf16)
            nc.sync.dma_start(out=owT_sb,
                              in_=owT_h[:, :].rearrange("p (c n) -> p c n", c=2))
            # duplicate rows 0:64 -> 64:128 (sbuf->sbuf on scalar queue)
            nc.scalar.dma_start(out=cos_sb[64:P, :], in_=cos_sb[0:64, :])
            nc.scalar.dma_start(out=sin_sb[64:P, :], in_=sin_sb[0:64, :])

            def xr(ts, c):
                return x_sb[ts][:, c, :]

            # per-t-slice result tiles (split for fine-grained deps)
            kd4 = [consts.tile([P, 512], bf16, tag=f"kd{ts}", name=f"kd{ts}")
                   for ts in range(TS)]
            qro = [[consts.tile([P, 512], bf16, tag=f"qro{m}{tb}",
                                name=f"qro{m}{tb}") for tb in range(TS)]
                   for m in range(2)]
            # v chunk stride padded to 72 elems (144B, 16B-aligned) for the
            # DMA xbar transpose destination
            v_sb4 = [consts.tile([P, 4, 72], bf16, tag=f"v{j}", name=f"v{j}")
                     for j in range(TS)]
            for j in range(TS):
                nc.vector.memset(v_sb4[j][:, :, HD : HD + 1], 1.0)
            # split per (tq, c-chunk) so out_proj's c=0 matmuls depend only
            # on the hp=0 unit's norm (exact subtile granularity)
            ot_sbc = [[consts.tile([P, 512], bf16, tag=f"ots{tq}{c}",
                                   name=f"ots{tq}{c}") for c in range(2)]
                      for tq in range(TS)]

            # ---- rope helpers (pair-adjacent d-layout, quadrant shuffle) ----
            def rope_q(qb, tsl, dst, fast=True):
                # dst = qb*cosF + swap(qb)*sinF, swap = pair exchange.
                # fast: muls on DVE (critical path / mid-attention lumps);
                # else gpsimd (idle during the head ramp before any norm)
                eng = nc.vector if fast else nc.gpsimd
                qsw = stg.tile([P, 512], bf16, tag="qsw", name="qsw")
                nc.vector.stream_shuffle(qsw, qb, SWAP_MASK)
                t1 = rope.tile([P, 512], bf16, tag="t1", name="t1")
                eng.tensor_mul(t1, qb, cos_sb[:, tsl])
                t2 = rope.tile([P, 512], bf16, tag="t2", name="t2")
                eng.tensor_mul(t2, qsw, sin_sb[:, tsl])
                nc.vector.tensor_add(dst[:, :], t1, t2)

            def rope_k(kb, tsl, dst, fast=False):
                # fast (ts0, on the critical path to QK(0)): muls on DVE
                # (bf16 2x, ~0.45us) instead of serial gpsimd (~1.15us each)
                eng = nc.vector if fast else nc.gpsimd
                ksw = stg.tile([64, 512], bf16, tag="ksw", name="ksw")
                nc.vector.stream_shuffle(ksw, kb, SWAP_MASK)
                t1 = rope.tile([64, 512], bf16, tag="kt1", name="kt1")
                eng.tensor_mul(t1, kb, cos_sb[0:64, tsl])
                t2 = rope.tile([64, 512], bf16, tag="kt2", name="kt2")
                eng.tensor_mul(t2, ksw, sin_sb[0:64, tsl])
                # write k and its partition-64 duplicate (row-tiled QK pair)
                nc.vector.tensor_add(dst[0:64, :], t1, t2)
                nc.vector.tensor_add(dst[64:P, :], t1, t2)

            # ---- head: kv + q(m0) projection per t-slice ----
            def head_ts(ts):
                tsl = slice(ts * 512, (ts + 1) * 512)
                # interleave the two psum halves (different banks) so
                # consecutive matmuls pipeline instead of serializing on
                # same-bank accumulation
                pt = ps_sc.tile([P, 1024], f32, tag="sc", name=f"kvq{ts}")
                for c in range(KC):
                    nc.tensor.matmul(pt[:, 0:512], lhsT=kvw_sb[:, c, :],
                                     rhs=xr(ts, c), start=(c == 0),
                                     stop=(c == KC - 1))
                    nc.tensor.matmul(pt[:, 512:1024], lhsT=qw0_sb[:, c, :],
                                     rhs=xr(ts, c), start=(c == 0),
                                     stop=(c == KC - 1))
                # stage order: kb, qb first (both ropes gate QK(0)); vb last
                kb = stg.tile([64, 512], bf16, tag="kb", name="kb")
                nc.scalar.copy(kb, pt[0:64, 0:512])
                qb = stg.tile([P, 512], bf16, tag="qb", name="qb")
                nc.scalar.copy(qb, pt[:, 512:1024])
                vb = stg.tile([64, 512], bf16, tag="vb", name="vb")
                nc.scalar.copy(vb, pt[64:P, 0:512])
                rope_k(kb, tsl, kd4[ts], fast=(ts == 0))
                rope_q(qb, tsl, qro[0][ts])
                vtr = stg.tile([P, 4, HD], bf16, tag="vtr", name="vtr")
                nc.sync.dma_start_transpose(out=vtr[:, :, :], in_=vb)
                nc.vector.tensor_copy(v_sb4[ts][:, :, 0:HD], vtr)

            # PE warmup: dummy matmuls on a memset scratch while x streams in,
            # so the HAM clock-gate is at 8/8 when the real projections start
            scratch = consts.tile([P, 512], bf16)
            nc.vector.memset(scratch, 0.0)
            wps = ps_ot.tile([P, 512], f32, tag="ot", name="warm")
            for _ in range(20):
                nc.tensor.matmul(wps, lhsT=scratch[:, 0:P], rhs=scratch,
                                 start=True, stop=True)

            # ts2/ts3 are injected into the early attention slices so the
            # PE stream does not serialize on the x DMA tail
            head_ts(0)
            head_ts(1)

            # ---- q(m1) projection lumps, injected into the slice stream ----
            def q1_lump(l):
                pt = ps_sc.tile([P, 1024], f32, tag="sc", name=f"q1l{l}")
                for c in range(KC):
                    for tb in (2 * l, 2 * l + 1):
                        psl = slice((tb % 2) * 512, (tb % 2) * 512 + 512)
                        nc.tensor.matmul(pt[:, psl], lhsT=qw1_sb[:, c, :],
                                         rhs=xr(tb, c), start=(c == 0),
                                         stop=(c == KC - 1))
                for tb in (2 * l, 2 * l + 1):
                    psl = slice((tb % 2) * 512, (tb % 2) * 512 + 512)
                    qb = stg.tile([P, 512], bf16, tag="qb", name="qb1")
                    nc.scalar.copy(qb, pt[:, psl])
                    rope_q(qb, slice(tb * 512, (tb + 1) * 512), qro[1][tb])

            # ---- attention: units (hp, tq) tq-major; 16 slices (s) each ----
            scale = 1.0 / float(np.sqrt(HD))
            units = [(u % 2, u // 2) for u in range(8)]
            slices = [(u, s) for u in range(8) for s in range(SC)]
            ot_tiles = {}

            def emit_qk(i):
                u, s = slices[i]
                hp, tq = units[u]
                kd = kd4[s // 4]
                ssl = slice((s % 4) * P, (s % 4 + 1) * P)
                qr = qro[hp][tq]
                cur = ps_sc.tile([P, 1024], f32, tag="sc", name=f"qk{i}")
                nc.tensor.matmul(cur[:, 0:512], lhsT=kd[0:64, ssl],
                                 rhs=qr[0:64, :], start=True, stop=True)
                nc.tensor.matmul(cur[:, 512:1024], lhsT=kd[64:P, ssl],
                                 rhs=qr[64:P, :], start=True, stop=True)
                return cur

            def emit_norm(u, ot):
                hp, tq = units[u]
                of = norm.tile([HD + 1, 1024], f32, tag="of", name="of")
                nc.scalar.copy(of, ot[: HD + 1, :])
                # denom row to partition 0 via a DVE copy (plain sbuf copies
                # tolerate base-64 reads; recip/broadcast/psum reads do not)
                dn = norm.tile([1, 1024], f32, tag="dn", name="dn")
                nc.vector.tensor_copy(dn, of[HD : HD + 1, :])
                recip = norm.tile([1, 1024], f32, tag="recip", name="recip")
                nc.vector.reciprocal_approx_fast(recip, dn)
                rb = norm.tile([HD, 1024], f32, tag="rb", name="rb")
                nc.gpsimd.partition_broadcast(rb, recip)
                nc.vector.tensor_mul(ot_sbc[tq][hp][0:HD, :], of[0:HD, 0:512],
                                     rb[:, 0:512])
                nc.vector.tensor_mul(ot_sbc[tq][hp][HD:P, :], of[0:HD, 512:1024],
                                     rb[:, 512:1024])

            # ---- out_proj for one 512-col t-block, 2 oc chunks per call ----
            def oproj_mms(pt, tq, k, c):
                for j in range(2):
                    oc = k * 2 + j
                    osl = slice(oc * P, (oc + 1) * P)
                    psl = slice(j * 512, (j + 1) * 512)
                    nc.tensor.matmul(pt[:, psl], lhsT=owT_sb[:, c, osl],
                                     rhs=ot_sbc[tq][c][:, :],
                                     start=(c == 0), stop=(c == 1))

            def oproj_out(pt, tq, k):
                o_sb = outs.tile([P, 1024], bf16, tag="o", name="o")
                if k % 2 == 0:
                    nc.vector.tensor_copy(o_sb, pt)
                else:
                    nc.scalar.copy(o_sb, pt)
                nc.sync.dma_start(
                    out=outr[:, k * 2 : k * 2 + 2, tq * 512 : (tq + 1) * 512],
                    in_=o_sb[:, :].rearrange("p (j t) -> p j t", j=2),
                )

            def emit_oproj(tq, k):
                pt = ps_sc.tile([P, 1024], f32, tag="sc", name=f"op{tq}{k}")
                oproj_mms(pt, tq, k, 0)
                oproj_mms(pt, tq, k, 1)
                oproj_out(pt, tq, k)

            # injection schedule: i -> list of thunks
            inject = {2: [lambda: head_ts(2)], 5: [lambda: head_ts(3)],
                      8: [lambda: q1_lump(0)], 36: [lambda: q1_lump(1)]}
            for tq in range(3):
                for k, di in enumerate((10, 16, 24, 30)):
                    inject.setdefault(32 * (tq + 1) + di, []).append(
                        (lambda tq=tq, k=k: emit_oproj(tq, k))
                    )

            def emit_exp(i, cur):
                u, s = slices[i]
                if EXP_PAT[u][s]:
                    pt = pexp.tile([P, 1024], bf16, tag="p", name="p")
                    nc.scalar.activation(pt, cur, AF.Exp, scale=scale)
                    return pt[:, 0:512], pt[:, 512:1024]
                st = psch.tile([P, 1024], i16, tag="q", name="q")
                nc.vector.tensor_scalar(
                    out=st[:, :], in0=cur[:, :],
                    scalar1=SCH_A, scalar2=SCH_B,
                    op0=ALU.mult, op1=ALU.add,
                )
                return st[:, 0:512].bitcast(bf16), st[:, 512:1024].bitcast(bf16)

            def emit_pv(i, ot, p2):
                u, s = slices[i]
                vst = v_sb4[s // 4][:, s % 4, 0 : HD + 1]
                nc.tensor.matmul(ot[:, 0:512], lhsT=vst, rhs=p2[0],
                                 start=(s == 0), stop=(s == SC - 1))
                nc.tensor.matmul(ot[:, 512:1024], lhsT=vst, rhs=p2[1],
                                 start=(s == 0), stop=(s == SC - 1))

            # slices processed in pairs: exp x2, QK-prefetch x2, PV x2 --
            # halves the QK<->PV array-config transitions per slice
            qk_bufs = {j: emit_qk(j) for j in range(4)}
            for p in range(len(slices) // 2):
                i = 2 * p
                u, s = slices[i]
                for ii in (i, i + 1):
                    for th in inject.get(ii, ()):
                        th()
                if s == 0:
                    ot_tiles[u] = ps_ot.tile([HD + 1, 1024], f32, tag="ot",
                                             name=f"uot{u}")
                ot = ot_tiles[u]
                cur0 = qk_bufs.pop(i)
                cur1 = qk_bufs.pop(i + 1)
                p20 = emit_exp(i, cur0)
                p21 = emit_exp(i + 1, cur1)
                for j in (i + 4, i + 5):
                    if j < len(slices):
                        qk_bufs[j] = emit_qk(j)
                emit_pv(i, ot, p20)
                emit_pv(i + 1, ot, p21)
                if s + 1 == SC - 1:
                    emit_norm(u, ot)

            # keep the PE clock-gate warm through norm(unit 7)'s chain: dummy
            # matmuls into unit 7's freed ot banks (safe once its of-copy ran)
            wp2 = ps_ot.tile([P, 1024], f32, tag="ot", name="warm2")
            for w in range(10):
                psl = slice((w % 2) * 512, (w % 2) * 512 + 512)
                nc.tensor.matmul(wp2[:, psl], lhsT=scratch[:, 0:P], rhs=scratch,
                                 start=True, stop=True)

            # tail: tq=3 out_proj; c=0 chunks depend only on norm(unit 6)
            # so their matmuls overlap norm(unit 7)'s chain and keep the
            # PE warm through the tail
            tail_pt = []
            for k in range(3):
                pt = ps_sc.tile([P, 1024], f32, tag="sc", name=f"op3{k}")
                tail_pt.append(pt)
                oproj_mms(pt, 3, k, 0)
            for k in range(3):
                oproj_mms(tail_pt[k], 3, k, 1)
                oproj_out(tail_pt[k], 3, k)
            emit_oproj(3, 3)

            if debug:
                for ts in range(TS):
                    nc.sync.dma_start(
                        out=dbg_kd_h[:, ts * 512 : (ts + 1) * 512], in_=kd4[ts])
                for j in range(TS):
                    nc.sync.dma_start(
                        out=dbg_v_h[:, j * 288 : (j + 1) * 288],
                        in_=v_sb4[j][:, :, :].rearrange("p a b -> p (a b)"),
                    )
                for m in range(2):
                    for tb in range(TS):
                        nc.sync.dma_start(
                            out=dbg_qro_h[:, (m * 4 + tb) * 512 : (m * 4 + tb + 1) * 512],
                            in_=qro[m][tb],
                        )
                for tq in range(TS):
                    for c in range(2):
                        nc.sync.dma_start(
                            out=dbg_ot_h[:, tq * 1024 + c * 512 :
                                         tq * 1024 + (c + 1) * 512],
                            in_=ot_sbc[tq][c],
                        )

    nc.finalize()
    return nc


def _get_module():
    if "nc" not in _MODULE_CACHE:
        _MODULE_CACHE["nc"] = _build()
    return _MODULE_CACHE["nc"]


# pair-adjacent d-layout (natural): swap via stream_shuffle quadrant mask
_PERM64 = np.arange(HD)


def _host_freqs(freqs_cos, freqs_sin):
    import ml_dtypes
    bf = ml_dtypes.bfloat16
    cos = np.asarray(freqs_cos, dtype=np.float32)  # (T, 32)
    sin = np.asarray(freqs_sin, dtype=np.float32)
    c64 = np.repeat(cos.T, 2, axis=0)              # (64, T): row d = cos[d//2]
    s64 = np.empty((HD, T), dtype=np.float32)
    s64[0::2] = -sin.T
    s64[1::2] = sin.T
    return np.ascontiguousarray(c64).astype(bf), np.ascontiguousarray(s64).astype(bf)


def kernel(x, q_w, kv_w, out_w, freqs_cos, freqs_sin):
    global LAST_RESULT
    import ml_dtypes
    from concourse.bass_utils import run_bass_kernel_spmd

    bf = ml_dtypes.bfloat16
    x = np.asarray(x, dtype=np.float32)
    q_w = np.asarray(q_w, dtype=np.float32)
    kv_w = np.asarray(kv_w, dtype=np.float32)
    out_w = np.asarray(out_w, dtype=np.float32)
    cosC, sinC = _host_freqs(freqs_cos, freqs_sin)

    def tile_cp(wT):
        # (D, M) -> (P, KC*M): row p holds [c0 | c1 | ...] chunks of col-block
        M = wT.shape[1]
        return np.ascontiguousarray(
            wT.reshape(KC, P, M).transpose(1, 0, 2).reshape(P, KC * M)
        ).astype(bf)

    # x pre-tiled: (TS*P, KC*512); row ts*P+p holds [c, t'] contiguous
    xts = []
    for b in range(B):
        xT = x[b].T.reshape(KC, P, TS, 512)          # (c, p, ts, t')
        xts.append(np.ascontiguousarray(
            xT.transpose(2, 1, 0, 3).reshape(TS * P, KC * 512)).astype(bf))

    in_maps = []
    for core in range(8):
        b, g = core // KV, core % KV
        # q rows for this group, block-permuted per head
        qrows = np.concatenate(
            [g * GH + h * HD + _PERM64 for h in range(REP)])
        qp = q_w[qrows, :]                      # (256, D)
        kT = kv_w[g * HD : (g + 1) * HD, :][_PERM64, :].T      # (D, 64)
        vT = kv_w[(KV + g) * HD : (KV + g + 1) * HD, :].T      # (D, 64)
        owT = out_w[:, g * GH : (g + 1) * GH].T                # (256, D)
        in_maps.append(
            dict(
                xts=xts[b],
                w01T=np.concatenate(
                    [tile_cp(np.concatenate([kT, vT], axis=1)),
                     tile_cp(qp[0:P].T)], axis=1),
                qw1T=tile_cp(qp[P : 2 * P].T),
                owT=np.ascontiguousarray(
                    owT.reshape(2, P, D).transpose(1, 0, 2).reshape(P, 2 * D)
                ).astype(bf),
                cosC=cosC,
                sinC=sinC,
            )
        )

    nc = _get_module()
    trace = os.environ.get("KERNEL_TRACE", "0") == "1"
    res = run_bass_kernel_spmd(nc, in_maps, core_ids=list(range(8)), trace=trace)
    LAST_RESULT = res

    out = np.zeros((B, T, D), dtype=np.float32)
    for core in range(8):
        b = core // KV
        out[b] += res.results[core]["outT"].T.astype(np.float32)
    return out
